# revision 46
# baseline (speedup 1.0000x reference)
"""Trainium2 Bass kernel for a dense transformer block (B=4, N=1024, D=1024,
H=16, Dh=64, MLP 4x), distributed over 8 NeuronCores with ZERO collectives.

Sharding: core c handles batch b = c//2, sequence half = c%2 (512 query
rows).  K/V are computed for the batch's full 1024-token sequence on both
cores of a pair (duplicated K/V FLOPs are far cheaper than the AllReduce a
tensor-parallel split would need).  The sequence is rotated per-core so the
core's own 512 rows are rows 0..511 of its input — attention is
permutation-invariant over keys, so all 8 cores run one identical SPMD
program.

Key implementation points:
- All weights AND constant layouts are cast/pre-tiled ON THE HOST
  (make_in_maps): bf16 weights (halves DMA traffic, zero device casting),
  biases pre-rearranged / pre-broadcast so every DMA is contiguous (the
  strided gather/broadcast descriptor DMAs measured 4-17us each).
  LayerNorm gamma/beta are folded into Wq/Wk/Wv/Wfc and their biases.
- LN runs in natural layout (DVE bn_stats; rstd = ACT sqrt of an exact DVE
  reciprocal); normalized activations are PE-transposed into the [d, seq]
  domain with one full PSUM bank (8 tiles) per copy-back pair.
- Scores: both heads of a pair run as CONCURRENT K=64 row-tiled matmuls
  (tile_position from base_partition 0/64) into the 2 banks of one
  [P, 2, 512] PSUM tile; a single ACT exp covers both banks.
- Softmax denominators ride a ones-column appended to V; the reciprocal is
  ACT Ln + Exp(-x) on both heads at once (partitions 0/32), broadcast via
  an e0/e1 bf16 matmul.  ACT exp is the attention pacer, so the K
  projections for ot=4..7 are computed INSIDE the attention loop (spread
  over the even head-pairs) where the PE otherwise idles.
- LN2 is pipelined per row-block between the Wo matmul groups.
- Dummy matmuls on an identity stationary keep the PE HAM clock-gate warm
  through the LN phases; ACT function tables are preloaded from idle spots.
- Wproj prefetches (chunked, on the sync HWDGE ring) during the Wo/LN2/FC
  phases; Wfc streams on the ACT HWDGE ring so the two never queue behind
  each other.
"""

import numpy as np

import bass_rust
import concourse.bass as bass
import concourse.mybir as mybir
import concourse.tile as tile
from concourse.masks import make_identity

F32 = mybir.dt.float32
BF16 = mybir.dt.bfloat16
AF = mybir.ActivationFunctionType
ALU = mybir.AluOpType

P = 128
D = 1024
S = 1024          # full sequence (per batch)
SO = 512          # own rows per core
H = 16
DH = 64
F = 4096
EPS = 1e-5
N_CORES = 8

ND = D // P       # 8   d tiles
NS = S // P       # 8   full-seq tiles
NSO = SO // P     # 4   own-seq tiles
NF = F // P       # 32  ff tiles
NFC_PRE = 6       # Wfc tiles prefetched before the FC loop


# --------------------------------------------------------------------------
# Workaround: this compiler build supports only ONE semaphore wait per
# instruction.  Move excess waits onto fresh NOPs inserted just before the
# offending instruction on the same engine.
# --------------------------------------------------------------------------
_counter = [0]


def _split_multiwaits(nc):
    nsplit = 0
    for fn in nc.m.functions:
        for blk in fn.blocks:
            il = list(blk.instructions)
            out = []
            changed = False
            for inst in il:
                si = inst.sync_info
                if si is not None and len(si.on_wait) > 1:
                    waits = list(si.on_wait)
                    for w in waits[:-1]:
                        _counter[0] += 1
                        nop = mybir.InstNoOp(
                            name=f"I-waitsplit-{_counter[0]}", ins=[], outs=[]
                        )
                        nop.engine = inst.engine
                        nop.sync_info = bass_rust.SyncInfo(on_wait=[w], on_update=[])
                        out.append(nop)
                        nc.register_instruction(nop, overwrite=True)
                    inst.sync_info = bass_rust.SyncInfo(
                        on_wait=[waits[-1]], on_update=list(si.on_update)
                    )
                    changed = True
                    nsplit += 1
                out.append(inst)
            if changed:
                blk.instructions = out
    return nsplit


def build():
    nc = bass.Bass(name="tfblock")

    x_ext = nc.declare_dram_parameter("x", [S, D], F32, isOutput=False)
    Wq_e = nc.declare_dram_parameter("Wq", [P, ND * D], BF16, isOutput=False)
    Wk_e = nc.declare_dram_parameter("Wk", [P, ND * D], BF16, isOutput=False)
    Wv_e = nc.declare_dram_parameter("Wv", [P, ND * D], BF16, isOutput=False)
    Wo_e = nc.declare_dram_parameter("Wo", [P, ND * D], BF16, isOutput=False)
    Wfc_e = nc.declare_dram_parameter("Wfc", [NF * P, ND * P], BF16, isOutput=False)
    Wp_e = nc.declare_dram_parameter("Wproj", [P, NF * D], BF16, isOutput=False)
    bq_e = nc.declare_dram_parameter("bq", [P, ND], F32, isOutput=False)
    bk_e = nc.declare_dram_parameter("bk", [P, ND], F32, isOutput=False)
    bfc_e = nc.declare_dram_parameter("bfc", [P, NF], F32, isOutput=False)
    bv_e = nc.declare_dram_parameter("bv", [P, D], F32, isOutput=False)
    bo_e = nc.declare_dram_parameter("bo", [P, D], F32, isOutput=False)
    bp_e = nc.declare_dram_parameter("bproj", [P, D], F32, isOutput=False)
    out_ext = nc.declare_dram_parameter("out", [SO, D], F32, isOutput=True)

    with tile.TileContext(nc) as tc:
        from contextlib import ExitStack

        with ExitStack() as top:
            consts = top.enter_context(tc.tile_pool(name="consts", bufs=1))
            persist = top.enter_context(tc.tile_pool(name="persist", bufs=1))

            # ---------------- pool stack (LIFO release discipline)
            x1N = persist.tile([P, NSO, D], F32, name="x1N")

            xown_cm = tc.tile_pool(name="xown", bufs=1)
            xown = xown_cm.__enter__()
            xN_own = xown.tile([P, NSO, D], F32, name="xN_own")

            wo_cm = tc.tile_pool(name="wop", bufs=1)
            wop = wo_cm.__enter__()
            Wo_sb = wop.tile([P, ND, D], BF16, name="Wo_sb")

            otp_cm = tc.tile_pool(name="otp", bufs=1)
            otp = otp_cm.__enter__()
            OT = otp.tile([P, ND, SO], BF16, name="OT")

            qkv_cm = tc.tile_pool(name="qkvp", bufs=1)
            qkvp = qkv_cm.__enter__()
            QT = qkvp.tile([P, ND, SO], BF16, name="QT")
            KT = qkvp.tile([P, ND, S], BF16, name="KT")
            VN = qkvp.tile([P, NS, H, DH + 1], BF16, name="VN")

            wk_cm = tc.tile_pool(name="wkp", bufs=1)
            wkp = wk_cm.__enter__()
            Wk_sb = wkp.tile([P, ND, D], BF16, name="Wk_sb")

            hT_cm = tc.tile_pool(name="hTp", bufs=1)
            hTp = hT_cm.__enter__()
            hT_own = hTp.tile([P, ND, SO], BF16, name="hT_own")
            hT_oth = hTp.tile([P, ND, SO], BF16, name="hT_oth")

            wv_cm = tc.tile_pool(name="wvp", bufs=1)
            wvp = wv_cm.__enter__()
            Wv_sb = wvp.tile([P, ND, D], BF16, name="Wv_sb")

            wq_cm = tc.tile_pool(name="wqp", bufs=1)
            wqp = wq_cm.__enter__()
            Wq_sb = wqp.tile([P, ND, D], BF16, name="Wq_sb")

            xoth_cm = tc.tile_pool(name="xoth", bufs=1)
            xoth = xoth_cm.__enter__()
            xN_oth = xoth.tile([P, NSO, D], F32, name="xN_oth")

            # ---------------- constants (ACT HWDGE ring: never queue behind
            # the big weight DMAs on the sync ring; all host-prepacked so
            # every transfer is contiguous)
            bq_t = consts.tile([P, ND], F32, name="bq_t")
            nc.scalar.dma_start(out=bq_t[:], in_=bq_e[:])
            bk_t = consts.tile([P, ND], F32, name="bk_t")
            nc.scalar.dma_start(out=bk_t[:], in_=bk_e[:])
            bfc_t = consts.tile([P, NF], F32, name="bfc_t")
            nc.scalar.dma_start(out=bfc_t[:], in_=bfc_e[:])
            bv_bc = consts.tile([P, D], F32, name="bv_bc")
            nc.scalar.dma_start(out=bv_bc[:], in_=bv_e[:])
            bo_bc = consts.tile([P, D], F32, name="bo_bc")
            nc.scalar.dma_start(out=bo_bc[:], in_=bo_e[:])
            bp_bc = consts.tile([P, D], F32, name="bp_bc")
            nc.scalar.dma_start(out=bp_bc[:], in_=bp_e[:])

            # ---------------- big DMA kickoff, priority order (sync ring)
            for st in range(NSO):
                nc.sync.dma_start(
                    out=xN_own[:, st, :], in_=x_ext[st * P : (st + 1) * P, :]
                )
            nc.sync.dma_start(
                out=Wq_sb[:], in_=Wq_e[:].rearrange("p (t d) -> p t d", t=ND)
            )
            for st in range(NSO):
                nc.sync.dma_start(
                    out=xN_oth[:, st, :], in_=x_ext[SO + st * P : SO + (st + 1) * P, :]
                )
            nc.sync.dma_start(
                out=Wk_sb[:], in_=Wk_e[:].rearrange("p (t d) -> p t d", t=ND)
            )
            nc.sync.dma_start(
                out=Wv_sb[:], in_=Wv_e[:].rearrange("p (t d) -> p t d", t=ND)
            )
            nc.sync.dma_start(
                out=Wo_sb[:], in_=Wo_e[:].rearrange("p (t d) -> p t d", t=ND)
            )

            eps_t = consts.tile([P, 1], F32, name="eps")
            nc.vector.memset(eps_t[:], EPS)
            e0 = consts.tile([P, P], BF16, name="e0")
            nc.gpsimd.memset(e0[:], 0.0)
            nc.gpsimd.memset(e0[0:1, :], 1.0)
            e1 = consts.tile([P, P], BF16, name="e1")
            nc.gpsimd.memset(e1[:], 0.0)
            nc.gpsimd.memset(e1[32:33, :], 1.0)
            ident = consts.tile([P, P], BF16, name="ident")
            make_identity(nc, ident[:])
            warm_mv = consts.tile([P, SO], BF16, name="warm_mv")
            nc.gpsimd.memset(warm_mv[:], 0.0)
            scr1 = consts.tile([P, 1], F32, name="scr1")
            nc.vector.memset(VN[:, :, :, DH : DH + 1], 1.0)

            # ==================================================== LN1 + QKV
            phA = ExitStack()
            psA = phA.enter_context(tc.tile_pool(name="psA", bufs=1, space="PSUM"))
            lnp = phA.enter_context(tc.tile_pool(name="ln1", bufs=3))
            statp = phA.enter_context(tc.tile_pool(name="stat1", bufs=1))

            def warm(n, pool):
                for _ in range(n):
                    ps = pool.tile([P, SO], F32, tag="warm", bufs=1)
                    nc.tensor.matmul(ps[:], ident[:], warm_mv[:], start=True, stop=True)

            warm(14, psA)

            mvN = statp.tile([P, NS, 2], F32, name="mvN")
            rstdN = statp.tile([P, NS, 1], F32, name="rstdN")
            nbN = statp.tile([P, NS, 1], F32, name="nbN")

            def ln_stats_tile(xsrc, i, st):
                stats = lnp.tile([P, 2, 6], F32, tag="st1")
                for g in range(2):
                    nc.vector.bn_stats(
                        out=stats[:, g, :], in_=xsrc[:, i, g * 512 : (g + 1) * 512]
                    )
                nc.vector.bn_aggr(out=mvN[:, st, :], in_=stats[:])

            def ln_rstd_batch(base):
                # batched rstd / neg-bias for 4 tiles
                ve = lnp.tile([P, NSO, 1], F32, tag="ve1")
                nc.vector.tensor_scalar(
                    ve[:], mvN[:, base : base + NSO, 1:2], eps_t[:], None, ALU.add
                )
                rv = lnp.tile([P, NSO, 1], F32, tag="rv1")
                nc.vector.reciprocal(out=rv[:], in_=ve[:])
                nc.scalar.activation(
                    out=rstdN[:, base : base + NSO, :], in_=rv[:], func=AF.Sqrt
                )
                nc.vector.tensor_tensor(
                    nbN[:, base : base + NSO, :],
                    mvN[:, base : base + NSO, 0:1],
                    rstdN[:, base : base + NSO, :],
                    ALU.mult,
                )
                nc.vector.tensor_scalar(
                    nbN[:, base : base + NSO, :],
                    nbN[:, base : base + NSO, :],
                    -1.0,
                    None,
                    ALU.mult,
                )

            def ln_apply_transpose(xsrc, i, st, hTx, nwarm):
                hn = lnp.tile([P, D], BF16, tag="hn1")
                nc.scalar.activation(
                    out=hn[:],
                    in_=xsrc[:, i, :],
                    func=AF.Identity,
                    bias=nbN[:, st, :],
                    scale=rstdN[:, st, :],
                )
                trp = psA.tile([P, ND, P], BF16, tag="tr", bufs=3)
                for dt in range(ND):
                    nc.tensor.transpose(
                        trp[:, dt, :], hn[:, dt * P : (dt + 1) * P], ident[:]
                    )
                nc.vector.tensor_copy(
                    out=hTx[:, 0:4, i * P : (i + 1) * P], in_=trp[:, 0:4, :]
                )
                nc.scalar.copy(
                    out=hTx[:, 4:8, i * P : (i + 1) * P], in_=trp[:, 4:8, :]
                )
                if nwarm:
                    warm(nwarm, psA)

            for i in range(NSO):
                ln_stats_tile(xN_own, i, i)
            ln_rstd_batch(0)
            for i in range(NSO):
                ln_apply_transpose(xN_own, i, i, hT_own, 2)

            # Q projection (own rows only): QT[dq, q] in transposed layout.
            # The other-half LN stats interleave with the Q bias drains on
            # the DVE queue so neither blocks the other.
            for ot in range(ND):
                ps = psA.tile([P, SO], F32, tag="acc", bufs=4)
                for kt in range(ND):
                    nc.tensor.matmul(
                        ps[:],
                        Wq_sb[:, kt, ot * P : (ot + 1) * P],
                        hT_own[:, kt, :],
                        start=(kt == 0),
                        stop=(kt == ND - 1),
                    )
                nc.vector.tensor_scalar(
                    QT[:, ot, :], ps[:], bq_t[:, ot : ot + 1], None, ALU.add
                )
                if ot % 2 == 1:
                    ln_stats_tile(xN_oth, ot // 2, NSO + ot // 2)
            ln_rstd_batch(NSO)

            for i in range(NSO):
                ln_apply_transpose(xN_oth, i, NSO + i, hT_oth, 1 if i % 2 == 0 else 0)

            # preload the Ln/Exp ACT table set while the PE is busy (the
            # attention phase needs it; loading costs 1.3us of ACT)
            nc.scalar.activation(out=scr1[:], in_=eps_t[:], func=AF.Exp)

            # K for ot 0..3 (4..7 are computed inside the attention loop):
            # stationary Wk tile reused for both seq halves
            for ot in range(NSO):
                ps0 = psA.tile([P, SO], F32, tag="acc", bufs=4)
                ps1 = psA.tile([P, SO], F32, tag="acc", bufs=4)
                for kt in range(ND):
                    w = Wk_sb[:, kt, ot * P : (ot + 1) * P]
                    nc.tensor.matmul(
                        ps0[:], w, hT_own[:, kt, :], start=(kt == 0), stop=(kt == ND - 1)
                    )
                    nc.tensor.matmul(
                        ps1[:], w, hT_oth[:, kt, :], start=(kt == 0), stop=(kt == ND - 1)
                    )
                nc.vector.tensor_scalar(
                    KT[:, ot, 0:SO], ps0[:], bk_t[:, ot : ot + 1], None, ALU.add
                )
                nc.vector.tensor_scalar(
                    KT[:, ot, SO:S], ps1[:], bk_t[:, ot : ot + 1], None, ALU.add
                )

            # V natural rows per key tile st, heads oh*8..oh*8+7
            for oh in range(2):
                for st in range(NS):
                    hTx = hT_own if st < NSO else hT_oth
                    st4 = st % NSO
                    ps = psA.tile([P, SO], F32, tag="acc", bufs=4)
                    for kt in range(ND):
                        nc.tensor.matmul(
                            ps[:],
                            hTx[:, kt, st4 * P : (st4 + 1) * P],
                            Wv_sb[:, kt, oh * SO : (oh + 1) * SO],
                            start=(kt == 0),
                            stop=(kt == ND - 1),
                        )
                    nc.vector.tensor_tensor(
                        VN[:, st, oh * 8 : (oh + 1) * 8, 0:DH],
                        ps[:].rearrange("p (h e) -> p h e", h=8),
                        bv_bc[:, oh * SO : (oh + 1) * SO].rearrange(
                            "p (h e) -> p h e", h=8
                        ),
                        ALU.add,
                    )

            # pre-bias the residual with bo (x + bo), in place
            for st in range(NSO):
                nc.vector.tensor_tensor(
                    xN_own[:, st, :], xN_own[:, st, :], bo_bc[:], ALU.add
                )

            phA.close()
            xoth_cm.__exit__(None, None, None)
            wq_cm.__exit__(None, None, None)
            wv_cm.__exit__(None, None, None)

            # ==================================================== attention
            phB = ExitStack()
            attn = phB.enter_context(tc.tile_pool(name="attn", bufs=1))
            ssp = phB.enter_context(tc.tile_pool(name="ssp", bufs=1, space="PSUM"))
            pop = phB.enter_context(tc.tile_pool(name="pop", bufs=1, space="PSUM"))

            rec16 = attn.tile([P, SO], BF16, name="rec16")
            nc.gpsimd.memset(rec16[:], 0.0)
            ddp = attn.tile([DH, SO], F32, name="ddp")
            nc.gpsimd.memset(ddp[:], 1.0)

            def norm_pre(j, po_pair):
                # 1/denominator for both heads: pack the two denominator rows
                # at partitions 0 and 32 (32-aligned partition bases), then
                # one Ln + one Exp(-x) on ACT covering both
                nc.vector.tensor_copy(out=ddp[0:1, :], in_=po_pair[0][DH : DH + 1, :])
                nc.vector.tensor_copy(out=ddp[32:33, :], in_=po_pair[1][DH : DH + 1, :])
                lnr = attn.tile([DH, SO], F32, tag="lnr", bufs=2)
                nc.scalar.activation(out=lnr[:], in_=ddp[:], func=AF.Ln)
                nc.scalar.activation(
                    out=rec16[0:DH, :], in_=lnr[:], func=AF.Exp, scale=-1.0
                )

            def norm_mid_post(j, po_pair):
                # broadcast 1/denom to 64 partitions via e0/e1 matmul (PSUM),
                # bounce to SBUF (DVE reads at most one PSUM operand), then
                # normalize the attention output into OT
                for hp in range(2):
                    bc = pop.tile([P, SO], F32, tag="iacc", bufs=2)
                    nc.tensor.matmul(
                        bc[0:DH, :],
                        (e0 if hp == 0 else e1)[:, 0:DH],
                        rec16[:],
                        start=True,
                        stop=True,
                    )
                    bcs = attn.tile([DH, SO], BF16, tag="bcs", bufs=2)
                    nc.vector.tensor_copy(out=bcs[:], in_=bc[0:DH, :])
                    nc.vector.tensor_tensor(
                        OT[hp * DH : (hp + 1) * DH, j, :],
                        po_pair[hp][0:DH, :],
                        bcs[:],
                        ALU.mult,
                    )

            prev = None  # (j, (po_e, po_o))
            for j in range(H // 2):
                pr = attn.tile([P, NS, 2, SO], BF16, tag="pr", bufs=2)
                po_e = pop.tile([P, SO], F32, tag="po", bufs=2)
                po_o = pop.tile([P, SO], F32, tag="po", bufs=2)
                po = (po_e, po_o)
                if prev is not None:
                    norm_pre(*prev)
                # K block hidden in this head-pair's PE slack (even j only)
                kb_ot = 4 + j // 2 if j % 2 == 0 else None
                kps = None
                for kb in range(NS):
                    ss = ssp.tile([P, 2, SO], F32, tag="ss", bufs=2)
                    for hp in range(2):
                        nc.tensor.matmul(
                            ss[:, hp, :],
                            KT[hp * DH : (hp + 1) * DH, j, kb * P : (kb + 1) * P],
                            QT[hp * DH : (hp + 1) * DH, j, :],
                            start=True,
                            stop=True,
                        )
                    nc.scalar.activation(
                        out=pr[:, kb, :, :], in_=ss[:], func=AF.Exp, scale=0.125
                    )
                    if kb == 1 and prev is not None:
                        norm_mid_post(*prev)
                        prev = None
                    if kb >= 1:
                        kp = kb - 1
                        for hp in range(2):
                            nc.tensor.matmul(
                                po[hp][0 : DH + 1, :],
                                VN[:, kp, 2 * j + hp, :],
                                pr[:, kp, hp, :],
                                start=(kp == 0),
                                stop=False,
                            )
                    if kb_ot is not None and 2 <= kb <= 5:
                        if kb == 2:
                            kps0 = pop.tile([P, SO], F32, tag="iacc", bufs=2)
                            kps1 = pop.tile([P, SO], F32, tag="iacc", bufs=2)
                            kps = (kps0, kps1)
                        for kt in (2 * (kb - 2), 2 * (kb - 2) + 1):
                            w = Wk_sb[:, kt, kb_ot * P : (kb_ot + 1) * P]
                            nc.tensor.matmul(
                                kps[0][:],
                                w,
                                hT_own[:, kt, :],
                                start=(kt == 0),
                                stop=(kt == ND - 1),
                            )
                            nc.tensor.matmul(
                                kps[1][:],
                                w,
                                hT_oth[:, kt, :],
                                start=(kt == 0),
                                stop=(kt == ND - 1),
                            )
                    if kb_ot is not None and kb == 6:
                        nc.vector.tensor_scalar(
                            KT[:, kb_ot, 0:SO],
                            kps[0][:],
                            bk_t[:, kb_ot : kb_ot + 1],
                            None,
                            ALU.add,
                        )
                        nc.vector.tensor_scalar(
                            KT[:, kb_ot, SO:S],
                            kps[1][:],
                            bk_t[:, kb_ot : kb_ot + 1],
                            None,
                            ALU.add,
                        )
                kp = NS - 1
                for hp in range(2):
                    nc.tensor.matmul(
                        po[hp][0 : DH + 1, :],
                        VN[:, kp, 2 * j + hp, :],
                        pr[:, kp, hp, :],
                        start=False,
                        stop=True,
                    )
                prev = (j, po)

            norm_pre(*prev)
            norm_mid_post(*prev)

            phB.close()
            hT_cm.__exit__(None, None, None)
            wk_cm.__exit__(None, None, None)
            qkv_cm.__exit__(None, None, None)

            # ============================================ Wo + residual + LN2
            # Wproj prefetches (chunked) during this phase and the FC loop
            wpF_cm = tc.tile_pool(name="wpF", bufs=1)
            wpF = wpF_cm.__enter__()
            WpF = wpF.tile([P, NF, D], BF16, name="WpF")
            for ch in range(4):
                nc.sync.dma_start(
                    out=WpF[:, ch * 8 : (ch + 1) * 8, :],
                    in_=Wp_e[:, ch * 8 * D : (ch + 1) * 8 * D].rearrange(
                        "p (t d) -> p t d", t=8
                    ),
                )

            stgF_cm = tc.tile_pool(name="stgF", bufs=NFC_PRE)
            stgF = stgF_cm.__enter__()

            def wfc_fetch(ft):
                t = stgF.tile([P, ND, P], BF16, tag="wfc")
                nc.scalar.dma_start(
                    out=t[:],
                    in_=Wfc_e[ft * P : (ft + 1) * P, :].rearrange(
                        "p (t d) -> p t d", t=ND
                    ),
                )
                return t

            wfc_pre = [wfc_fetch(ft) for ft in range(NFC_PRE)]

            mlp_cm = tc.tile_pool(name="mlpp", bufs=1)
            mlpp = mlp_cm.__enter__()
            h2T = mlpp.tile([P, ND, SO], BF16, name="h2T")

            phC = ExitStack()
            psC = phC.enter_context(tc.tile_pool(name="psC", bufs=1, space="PSUM"))
            ln2p = phC.enter_context(tc.tile_pool(name="ln2", bufs=3))
            stat2 = phC.enter_context(tc.tile_pool(name="stat2", bufs=1))

            mv2 = stat2.tile([P, NSO, 2], F32, name="mv2")
            rstd2 = stat2.tile([P, NSO, 1], F32, name="rstd2")
            nb2 = stat2.tile([P, NSO, 1], F32, name="nb2")

            def ln2_transpose(qb):
                trp = psC.tile([P, ND, P], BF16, tag="tr2", bufs=2)
                h2n = ln2_hn[qb]
                for dt in range(ND):
                    nc.tensor.transpose(
                        trp[:, dt, :], h2n[:, dt * P : (dt + 1) * P], ident[:]
                    )
                nc.vector.tensor_copy(
                    out=h2T[:, 0:4, qb * P : (qb + 1) * P], in_=trp[:, 0:4, :]
                )
                nc.scalar.copy(
                    out=h2T[:, 4:8, qb * P : (qb + 1) * P], in_=trp[:, 4:8, :]
                )
                # pre-bias the residual with bproj, in place (after LN2 read)
                nc.vector.tensor_tensor(
                    x1N[:, qb, :], x1N[:, qb, :], bp_bc[:], ALU.add
                )

            ln2_hn = {}
            for qb in range(NSO):
                ps0 = psC.tile([P, SO], F32, tag="oacc", bufs=4)
                ps1 = psC.tile([P, SO], F32, tag="oacc", bufs=4)
                for kt in range(ND):
                    o = OT[:, kt, qb * P : (qb + 1) * P]
                    nc.tensor.matmul(
                        ps0[:], o, Wo_sb[:, kt, 0:SO], start=(kt == 0), stop=(kt == ND - 1)
                    )
                    nc.tensor.matmul(
                        ps1[:], o, Wo_sb[:, kt, SO:D], start=(kt == 0), stop=(kt == ND - 1)
                    )
                nc.vector.tensor_tensor(
                    x1N[:, qb, 0:SO], xN_own[:, qb, 0:SO], ps0[:], ALU.add
                )
                nc.vector.tensor_tensor(
                    x1N[:, qb, SO:D], xN_own[:, qb, SO:D], ps1[:], ALU.add
                )
                # LN2 stats + rstd + apply for this row block (overlaps the
                # next Wo matmul group)
                stats = ln2p.tile([P, 2, 6], F32, tag="st2")
                for g in range(2):
                    nc.vector.bn_stats(
                        out=stats[:, g, :], in_=x1N[:, qb, g * 512 : (g + 1) * 512]
                    )
                nc.vector.bn_aggr(out=mv2[:, qb, :], in_=stats[:])
                ve2 = ln2p.tile([P, 1, 1], F32, tag="ve2")
                nc.vector.tensor_scalar(
                    ve2[:], mv2[:, qb : qb + 1, 1:2], eps_t[:], None, ALU.add
                )
                rv2 = ln2p.tile([P, 1, 1], F32, tag="rv2")
                nc.vector.reciprocal(out=rv2[:], in_=ve2[:])
                nc.scalar.activation(
                    out=rstd2[:, qb : qb + 1, :], in_=rv2[:], func=AF.Sqrt
                )
                nc.vector.tensor_tensor(
                    nb2[:, qb : qb + 1, :],
                    mv2[:, qb : qb + 1, 0:1],
                    rstd2[:, qb : qb + 1, :],
                    ALU.mult,
                )
                nc.vector.tensor_scalar(
                    nb2[:, qb : qb + 1, :], nb2[:, qb : qb + 1, :], -1.0, None, ALU.mult
                )
                h2n = ln2p.tile([P, D], BF16, tag="h2n", bufs=4)
                nc.scalar.activation(
                    out=h2n[:],
                    in_=x1N[:, qb, :],
                    func=AF.Identity,
                    bias=nb2[:, qb, :],
                    scale=rstd2[:, qb, :],
                )
                ln2_hn[qb] = h2n
                if qb >= 1:
                    ln2_transpose(qb - 1)
            ln2_transpose(NSO - 1)
            warm(4, psC)
            # preload the Gelu ACT table set before the FC loop needs it
            nc.scalar.activation(out=scr1[:], in_=eps_t[:], func=AF.Gelu)

            phC.close()

            # ==================================================== MLP
            phD = ExitStack()
            gtp = phD.enter_context(tc.tile_pool(name="gtp", bufs=1))
            psD = phD.enter_context(tc.tile_pool(name="psD", bufs=1, space="PSUM"))
            opool = phD.enter_context(tc.tile_pool(name="opool", bufs=3))

            GT = gtp.tile([P, NF, SO], BF16, name="GT")

            wfc_tiles = list(wfc_pre)
            for ft in range(NF):
                if ft + NFC_PRE < NF:
                    wfc_tiles.append(wfc_fetch(ft + NFC_PRE))
                wfc = wfc_tiles[ft]
                ps = psD.tile([P, SO], F32, tag="gacc", bufs=3)
                for kt in range(ND):
                    nc.tensor.matmul(
                        ps[:],
                        wfc[:, kt, :],
                        h2T[:, kt, :],
                        start=(kt == 0),
                        stop=(kt == ND - 1),
                    )
                nc.scalar.activation(
                    out=GT[:, ft, :], in_=ps[:], func=AF.Gelu, bias=bfc_t[:, ft : ft + 1]
                )

            # proj, NATURAL output, fused residual:
            # out[s, d] = (x1 + bproj)[s, d] + sum_ft GT[:,ft,s].T @ Wp[ft, d]
            for qb in range(NSO):
                ps0 = psD.tile([P, SO], F32, tag="pacc", bufs=4)
                ps1 = psD.tile([P, SO], F32, tag="pacc", bufs=4)
                for ft in range(NF):
                    g = GT[:, ft, qb * P : (qb + 1) * P]
                    nc.tensor.matmul(
                        ps0[:],
                        g,
                        WpF[:, ft, 0:SO],
                        start=(ft == 0),
                        stop=(ft == NF - 1),
                    )
                    nc.tensor.matmul(
                        ps1[:],
                        g,
                        WpF[:, ft, SO:D],
                        start=(ft == 0),
                        stop=(ft == NF - 1),
                    )
                for dh, ps in ((0, ps0), (1, ps1)):
                    of = opool.tile([P, SO], F32, tag="of")
                    nc.vector.tensor_tensor(
                        of[:], x1N[:, qb, dh * SO : (dh + 1) * SO], ps[:], ALU.add
                    )
                    nc.sync.dma_start(
                        out=out_ext[qb * P : (qb + 1) * P, dh * SO : (dh + 1) * SO],
                        in_=of[:],
                    )

            phD.close()
            mlp_cm.__exit__(None, None, None)
            stgF_cm.__exit__(None, None, None)
            wpF_cm.__exit__(None, None, None)
            otp_cm.__exit__(None, None, None)
            wo_cm.__exit__(None, None, None)
            xown_cm.__exit__(None, None, None)

    _split_multiwaits(nc)
    return nc


_NC_CACHE = None


def _get_nc():
    global _NC_CACHE
    if _NC_CACHE is None:
        _NC_CACHE = build()
    return _NC_CACHE


def make_in_maps(inputs):
    """Shard FULL inputs into per-core input maps (own rows rotated first).

    All weights are folded (LN gamma/beta), cast to bf16 and pre-tiled here;
    bias vectors are pre-rearranged/pre-broadcast so every device DMA is a
    fast contiguous transfer.
    """
    import ml_dtypes

    BF = ml_dtypes.bfloat16
    x = np.asarray(inputs["x"], dtype=np.float32)

    def f64(k):
        return np.asarray(inputs[k], dtype=np.float64)

    ln1w, ln1b = f64("ln1_w"), f64("ln1_b")
    ln2w, ln2b = f64("ln2_w"), f64("ln2_b")

    def fold(Wn, bn, lw, lb):
        W = f64(Wn)
        return W * lw[:, None], f64(bn) + lb @ W

    Wq, bq = fold("Wq", "bq", ln1w, ln1b)
    Wk, bk = fold("Wk", "bk", ln1w, ln1b)
    Wv, bv = fold("Wv", "bv", ln1w, ln1b)
    Wfc, bfc = fold("Wfc", "bfc", ln2w, ln2b)
    Wo, bo = f64("Wo"), f64("bo")
    Wp, bp = f64("Wproj"), f64("bproj")

    def rowtile(W):  # [T*P, M] -> [P, T*M] with [p, t*M+m] = W[t*P+p, m]
        T = W.shape[0] // P
        M = W.shape[1]
        return np.ascontiguousarray(
            W.reshape(T, P, M).transpose(1, 0, 2).reshape(P, T * M)
        ).astype(BF)

    # Wfc column-blocks: [ft*P+p, kt*P+j] = Wfc[kt*P+p, ft*P+j]
    WfcT = np.ascontiguousarray(
        Wfc.reshape(ND, P, NF, P).transpose(2, 1, 0, 3).reshape(NF * P, ND * P)
    ).astype(BF)

    def coltile(b):  # [T*P] -> [P, T] with [p, t] = b[t*P + p]
        T = b.shape[0] // P
        return np.ascontiguousarray(b.reshape(T, P).T).astype(np.float32)

    def bcast(b):  # [D] -> [P, D]
        return np.ascontiguousarray(
            np.broadcast_to(b.astype(np.float32), (P, b.shape[0]))
        )

    shared = {
        "Wq": rowtile(Wq),
        "Wk": rowtile(Wk),
        "Wv": rowtile(Wv),
        "Wo": rowtile(Wo),
        "Wfc": WfcT,
        "Wproj": rowtile(Wp),
        "bq": coltile(bq),
        "bk": coltile(bk),
        "bfc": coltile(bfc),
        "bv": bcast(bv),
        "bo": bcast(bo),
        "bproj": bcast(bp),
    }
    in_maps = []
    for c in range(N_CORES):
        b, half = c // 2, c % 2
        xb = x[b]
        x_core = np.concatenate(
            [xb[half * SO : (half + 1) * SO], xb[(1 - half) * SO : (2 - half) * SO]],
            axis=0,
        )
        m = {"x": np.ascontiguousarray(x_core)}
        m.update(shared)
        in_maps.append(m)
    return in_maps


def kernel(**inputs) -> np.ndarray:
    from concourse.bass_utils import run_bass_kernel_spmd

    nc = _get_nc()
    in_maps = make_in_maps(inputs)
    res = run_bass_kernel_spmd(nc, in_maps, list(range(N_CORES)))
    B = 4
    out = np.empty((B, S, D), dtype=np.float32)
    for c in range(N_CORES):
        b, half = c // 2, c % 2
        out[b, half * SO : (half + 1) * SO] = res.results[c]["out"]
    return out
